# revision 7
# baseline (speedup 1.0000x reference)
"""CSPN propagation step on 8 Trainium2 NeuronCores (pure batch data-parallel).

Math (algebraic collapse of the reference's fold(unfold) structure):
  abs_sum = sum_c |aff_c|;  raw_sum = sum_c aff_c;  r = 1/abs_sum;  n_c = aff_c * r
  S[y,x]  = sum_c n_c[y+rho_c, x+delta_c]   (zero outside the image)
  out     = cur * S + (1 - raw_sum) * coa
with per-channel tap offsets
  c:      0        1       2        3       4        5        6       7
  (rho,d) (+1,+1) (+1,0)  (+1,-1)  (0,+1)  (0,-1)  (-1,+1)  (-1,0)  (-1,-1)

Per core: 8 images, processed in 4 rounds of 2. Layout: partitions = y within a
128-row half, free = [channel][img*half block][x padded to 258]. Row shifts are
shifted-identity fp32r matmuls accumulating in PSUM (x shifts fold into AP column
offsets); channel reductions (raw/abs sums) are identity-weight fp32r matmuls over
ACT-staged fp32r copies; normalize + epilogue on vector/gpsimd. fp32r operands
must be written by a compute op (BIR verifier rejects raw DMA-fed fp32r matmuls),
hence the ACT rounding passes.
"""

import sys

sys.path.insert(0, "/opt/trn_rl_repo")

import numpy as np

from concourse import bass, bacc, mybir, tile
from concourse.bass_utils import run_bass_kernel_spmd

F32 = mybir.dt.float32
F32R = mybir.dt.float32r
ABS = mybir.ActivationFunctionType.Abs
COPY = mybir.ActivationFunctionType.Copy
H = W = 256
PB = 8  # images per core
IPR = 2  # images per round
NROUNDS = PB // IPR
WPAD = W + 2

# channel -> (row read offset rho, x read offset delta)
TAPS = {0: (1, 1), 1: (1, 0), 2: (1, -1), 3: (0, 1), 4: (0, -1),
        5: (-1, 1), 6: (-1, 0), 7: (-1, -1)}

# stationary-weight indices in the wmats input
W_NI0, W_I0, W_IP1, W_IM1, W_ETF, W_EBF = range(6)


def _wmats_np() -> np.ndarray:
    """[128, 6, 128] stationary matrices, indexed [k, which, m]; out[m] += W[k,m]*X[k]."""
    I = np.eye(128, dtype=np.float32)
    ip1 = np.eye(128, k=-1, dtype=np.float32)  # ones at [m+1, m]: out[m] += X[m+1]
    im1 = np.eye(128, k=1, dtype=np.float32)   # ones at [m-1, m]: out[m] += X[m-1]
    etf = np.zeros((128, 128), np.float32)
    etf[0, 127] = 1.0                          # out[127] += X[0]  (top-half fixup)
    ebf = np.zeros((128, 128), np.float32)
    ebf[127, 0] = 1.0                          # out[0] += X[127]  (bottom-half fixup)
    return np.stack([-I, I, ip1, im1, etf, ebf], axis=0).transpose(1, 0, 2).copy()


def build_program():
    nc = bacc.Bacc("TRN2", target_bir_lowering=False, debug=False)

    aff_d = nc.dram_tensor("affinity", [PB, 8, H, W], F32, kind="ExternalInput").ap()
    cur_d = nc.dram_tensor("cur", [PB, 1, H, W], F32, kind="ExternalInput").ap()
    coa_d = nc.dram_tensor("coa", [PB, 1, H, W], F32, kind="ExternalInput").ap()
    wm_d = nc.dram_tensor("wmats", [128, 6, 128], F32, kind="ExternalInput").ap()
    out_d = nc.dram_tensor("out", [PB, 1, H, W], F32, kind="ExternalOutput").ap()

    with tile.TileContext(nc) as tc:
        with (
            tc.tile_pool(name="wpool", bufs=1) as wpool,
            tc.tile_pool(name="affp", bufs=2) as affp,
            tc.tile_pool(name="npool", bufs=1) as npool,
            tc.tile_pool(name="absp", bufs=3) as absp,
            tc.tile_pool(name="rawp", bufs=3) as rawp,
            tc.tile_pool(name="rp", bufs=2) as rp,
            tc.tile_pool(name="segp", bufs=2) as segp,
            tc.tile_pool(name="outp", bufs=2) as outp,
            tc.tile_pool(name="psum", bufs=1, space="PSUM") as psp,
        ):
            wt = wpool.tile([128, 6, 128], F32)
            nc.sync.dma_start(out=wt[:], in_=wm_d[:])
            # fp32r matmul operands must be produced rounded -> round once on ACT
            wtr = wpool.tile([128, 6, 128], F32R)
            nc.scalar.activation(out=wtr[:], in_=wt[:], func=COPY)

            def mm(out_ap, widx, x_ap, start, stop):
                nc.tensor.matmul(
                    out=out_ap,
                    lhsT=wtr[:, widx, :],
                    rhs=x_ap,
                    start=start,
                    stop=stop,
                )

            for rnd in range(NROUNDS):
                b0 = rnd * IPR
                afft = affp.tile([128, 8, 2 * IPR, WPAD], F32, tag="aff")
                ntile = npool.tile([128, 8, 2 * IPR, WPAD], F32R, tag="n")
                curt = segp.tile([128, 2 * IPR, W], F32, tag="cur")
                coat = segp.tile([128, 2 * IPR, W], F32, tag="coa")
                rt = rp.tile([128, 2 * IPR, WPAD], F32, tag="r")
                ot = outp.tile([128, 2 * IPR, W], F32, tag="out")
                tmp = outp.tile([128, 2 * IPR, W], F32, tag="tmp")
                rawn = [psp.tile([128, 2, W], F32, tag=f"rawn{i}", name=f"rawn{i}_{rnd}")
                        for i in range(IPR)]
                abst = [psp.tile([128, 2, W], F32, tag=f"abs{i}", name=f"abs{i}_{rnd}")
                        for i in range(IPR)]
                St = [psp.tile([128, 2, W], F32, tag=f"S{i}", name=f"S{i}_{rnd}")
                      for i in range(IPR)]

                # ---- loads ----
                for i in range(IPR):
                    b = b0 + i
                    for h in range(2):
                        nc.sync.dma_start(
                            out=afft[:, :, 2 * i + h, 1 : 1 + W],
                            in_=aff_d[b, :, 128 * h : 128 * (h + 1), :].rearrange(
                                "c p x -> p c x"
                            ),
                        )
                    nc.sync.dma_start(
                        out=curt[:, 2 * i : 2 * i + 2, :],
                        in_=cur_d[b, 0].rearrange("(h p) x -> p h x", p=128),
                    )
                    nc.sync.dma_start(
                        out=coat[:, 2 * i : 2 * i + 2, :],
                        in_=coa_d[b, 0].rearrange("(h p) x -> p h x", p=128),
                    )
                # zero the x-pad columns of aff and r; the full-width normalize
                # mul then writes all of n (incl. zero pads) as rounded fp32r
                nc.gpsimd.memset(afft[:, :, :, 0], 0.0)
                nc.gpsimd.memset(afft[:, :, :, WPAD - 1], 0.0)
                nc.gpsimd.memset(rt[:, :, 0], 0.0)
                nc.gpsimd.memset(rt[:, :, WPAD - 1], 0.0)

                # ---- neg raw_sum (PSUM rawn = -sum_c aff_c) over ACT-rounded copies ----
                for c in range(8):
                    afr = rawp.tile([128, 2 * IPR, W], F32R, tag="afr", name=f"afr{rnd}_{c}")
                    nc.scalar.activation(out=afr[:], in_=afft[:, c, :, 1 : 1 + W], func=COPY)
                    for i in range(IPR):
                        mm(rawn[i][:], W_NI0, afr[:, 2 * i : 2 * i + 2, :],
                           start=(c == 0), stop=(c == 7))

                # ---- abs staging (ACT) + abs_sum (PSUM) ----
                for c in range(8):
                    ab = absp.tile([128, 2 * IPR, W], F32R, tag="ab", name=f"ab{rnd}_{c}")
                    nc.scalar.activation(out=ab[:], in_=afft[:, c, :, 1 : 1 + W], func=ABS)
                    for i in range(IPR):
                        mm(abst[i][:], W_I0, ab[:, 2 * i : 2 * i + 2, :],
                           start=(c == 0), stop=(c == 7))

                # ---- r = 1/abs_sum ----
                for i in range(IPR):
                    nc.vector.reciprocal_approx_fast(
                        out=rt[:, 2 * i : 2 * i + 2, 1 : 1 + W], in_=abst[i][:]
                    )

                # ---- n_c = aff_c * r (fp32r rounded on write, pads 0*0=0) ----
                for c in range(8):
                    eng = nc.gpsimd if c < 3 else nc.vector
                    eng.tensor_mul(
                        out=ntile[:, c, :, :],
                        in0=afft[:, c, :, :],
                        in1=rt[:],
                    )

                # ---- S: shifted-identity matmuls with PSUM accumulation ----
                wmap = {1: W_IP1, 0: W_I0, -1: W_IM1}
                for rho in (1, 0, -1):
                    for c, (rc, dlt) in TAPS.items():
                        if rc != rho:
                            continue
                        for i in range(IPR):
                            mm(St[i][:], wmap[rho],
                               ntile[:, c, 2 * i : 2 * i + 2, 1 + dlt : 1 + dlt + W],
                               start=(rho == 1 and c == 0), stop=False)
                # half-boundary fixups: row 127 of top half reads row 0 of bottom
                # half (channels with rho=+1); row 0 of bottom half reads row 127
                # of top half (rho=-1). Outer image rows need nothing (zero).
                for i in range(IPR):
                    for c in (0, 1, 2):
                        dlt = TAPS[c][1]
                        mm(St[i][:, 0, :], W_ETF,
                           ntile[:, c, 2 * i + 1, 1 + dlt : 1 + dlt + W],
                           start=False, stop=False)
                    for c in (5, 6, 7):
                        dlt = TAPS[c][1]
                        mm(St[i][:, 1, :], W_EBF,
                           ntile[:, c, 2 * i, 1 + dlt : 1 + dlt + W],
                           start=False, stop=(c == 7))

                # ---- epilogue: out = cur*S + (1 - raw_sum)*coa ----
                for i in range(IPR):
                    sl = slice(2 * i, 2 * i + 2)
                    nc.vector.scalar_tensor_tensor(
                        out=tmp[:, sl, :], in0=rawn[i][:], scalar=1.0,
                        in1=coat[:, sl, :],
                        op0=mybir.AluOpType.add, op1=mybir.AluOpType.mult,
                    )
                    nc.vector.tensor_mul(out=ot[:, sl, :], in0=curt[:, sl, :], in1=St[i][:])
                    nc.vector.tensor_add(out=ot[:, sl, :], in0=ot[:, sl, :], in1=tmp[:, sl, :])

                # ---- store ----
                for i in range(IPR):
                    nc.sync.dma_start(
                        out=out_d[b0 + i, 0].rearrange("(h p) x -> p h x", p=128),
                        in_=ot[:, 2 * i : 2 * i + 2, :],
                    )

    nc.compile()
    return nc


_PROG = None


def _get_prog():
    global _PROG
    if _PROG is None:
        _PROG = build_program()
    return _PROG


_WM = _wmats_np()


def kernel(affinity, current_segmentation, coarse_segmentation):
    affinity = np.ascontiguousarray(np.asarray(affinity, dtype=np.float32))
    cur = np.ascontiguousarray(np.asarray(current_segmentation, dtype=np.float32))
    coa = np.ascontiguousarray(np.asarray(coarse_segmentation, dtype=np.float32))
    B = affinity.shape[0]
    n_cores = 8
    per = B // n_cores
    assert per == PB, f"program built for {PB} images/core, got {per}"

    in_maps = []
    for ci in range(n_cores):
        sl = slice(ci * per, (ci + 1) * per)
        in_maps.append({
            "affinity": affinity[sl],
            "cur": cur[sl],
            "coa": coa[sl],
            "wmats": _WM,
        })
    res = run_bass_kernel_spmd(_get_prog(), in_maps, list(range(n_cores)))
    outs = [np.asarray(res.results[ci]["out"]) for ci in range(n_cores)]
    return np.concatenate(outs, axis=0).astype(np.float32)
